# revision 28
# baseline (speedup 1.0000x reference)
"""Fused self-attention kernel for Trainium2 (Bass/Tile), SPMD over 8 cores.

Math (per batch b):
    q = x @ Wq + bq ; k = x @ Wk + bk ; v = x @ Wv + bv          [T, C]
    scores[t, s] = k[t] . q[s]      (non-causal, unscaled)
    beta = softmax(scores, axis=s)
    attn[t] = sum_s beta[t, s] * v[s]
    out = gamma * attn + x

Sharding: 8 cores = 4 batches x 2 halves of the output rows t. Each core's
x is rotated so its local 2048 output rows come first (softmax over s is
permutation invariant, so rotating s is safe).

Host-side layout prep (inside kernel(), pure numpy): x is transposed,
ones-augmented and cast to bf16 (xt), the residual slice is pre-tiled
to the SBUF partition layout (xr), and the weights are padded to
[128,128] bf16 with the bias folded in as row 64 (the ones row of xt
multiplies it back in).  gamma is folded into Wv/bv (the ones column of
v_aug that produces the softmax denominator is NOT scaled), so the
device kernel never sees gamma and out = num/denom + x directly.

On-chip: scoresT[s, t] with s on partitions; denominator via the ones
column of v_aug.  No max-subtraction (|scores| < ~60 for normalized
inputs; exp in fp32, overflow at 88).  exp is split between the Scalar
engine (table exp) and the Vector engine (Schraudolph-style bf16 bit
trick: bitcast(round(x*184.665 + 16248.7)) ~= exp(x) to ~3%), since ACT
alone (1 elem/cycle/lane) would be the bottleneck.  The TxT score
matrix never touches HBM.
"""

import numpy as np
import ml_dtypes
from contextlib import ExitStack

import concourse.bass as bass
import concourse.tile as tile
from concourse import bacc, mybir
from concourse.bass_utils import run_bass_kernel_spmd

FP32 = mybir.dt.float32
BF16 = mybir.dt.bfloat16
I16 = mybir.dt.int16
AF = mybir.ActivationFunctionType
ALU = mybir.AluOpType

B, T, C = 4, 4096, 64
P = 128
HALVES = 2
N_CORES = B * HALVES
TL = T // HALVES      # local output rows per core (2048)
TB = 1024             # t-block width (one PSUM accumulator pair)
N_TB = TL // TB       # 2
SB = 512              # psum-bank-sized matmul free dim
NT = T // P           # 32 s-tiles

# Schraudolph exp in bf16 bits: exp(x) ~= bitcast_bf16(round(A*x + BB))
SCHRAUD_A = 128.0 / np.log(2.0)          # 184.6650
SCHRAUD_B = 16256.0 - 0.0573 * 128.0     # 16248.67 (balanced max rel err ~3%)

# which s-tiles' exp goes to the Vector engine (rest on Scalar engine);
# tuned so ACT (997ns/tile + finalize scales) and DVE (1224ns/tile +
# projection copies + finalize) land at similar totals, both under PE
DVE_EXP = set(range(1, NT, 3)) | {2, 14, 26, 30}  # 14 of 32 per t-block


def _emit(tc, ctx, xt_d, xr_d, wq_d, wk_d, wv_d, id_d, out_d):
    nc = tc.nc

    const = ctx.enter_context(tc.tile_pool(name="const", bufs=1))
    expp = ctx.enter_context(tc.tile_pool(name="expp", bufs=6))
    osbp = ctx.enter_context(tc.tile_pool(name="osbp", bufs=2))
    outp = ctx.enter_context(tc.tile_pool(name="outp", bufs=2))
    smallp = ctx.enter_context(tc.tile_pool(name="smallp", bufs=8))
    # PSUM (8 banks): a manually-rotated ring of 6 one-bank slots inside a
    # single [128, 6, 512] tile (scores matmuls target single-bank slots;
    # exp reads a full-width [128,1024] view over an ALIGNED slot pair, so
    # the per-instruction overhead of ACT/DVE is paid once per 1024, not
    # per 512).  Slots advance in pairs; Tile's range-based dependency
    # tracking gives the WAR serialization for slot reuse.  Scores run 2-3
    # s-tiles ahead of attn so exp latency never stalls the PE FIFO.  One
    # shared po accumulator region = 2 banks (tb=0/tb=1 are sequential).
    ps_s = ctx.enter_context(tc.tile_pool(name="ps_s", bufs=6, space="PSUM"))
    ps_o = ctx.enter_context(tc.tile_pool(name="ps_o", bufs=1, space="PSUM"))

    # ---- constants & DMAs ----------------------------------------------
    ident = const.tile([P, P], BF16, tag="ident")
    wq = const.tile([P, P], BF16, tag="wq")
    wk = const.tile([P, P], BF16, tag="wk")
    wv = const.tile([P, P], BF16, tag="wv")
    xt = const.tile([P, T], BF16, tag="xt")   # rows 0:64 x.T, 64 ones, 65: zeros
    qt = const.tile([P, T], BF16, tag="qt")       # q.T, all s
    kt = const.tile([P, TL], BF16, tag="kt")      # k.T, local t
    va = const.tile([P, NT, P], BF16, tag="va")   # v_aug per s-tile [s,c]

    # DMA order = first-use order: xt chunk 0 + wk feed kt_round(0), then
    # wq/wv for the other projections; the identity (finalize-only) and the
    # residual (finalize-only) go last.
    nc.sync.dma_start(xt[:, 0:SB], xt_d.ap()[:, 0:SB])
    nc.sync.dma_start(wk, wk_d.ap())
    nc.sync.dma_start(wq, wq_d.ap())
    nc.sync.dma_start(wv, wv_d.ap())
    nc.sync.dma_start(xt[:, SB:TB], xt_d.ap()[:, SB:TB])
    for i in range(1, 4):
        nc.sync.dma_start(xt[:, i * TB:(i + 1) * TB],
                          xt_d.ap()[:, i * TB:(i + 1) * TB])
    xr = const.tile([P, TL // P, C], FP32, tag="xr")  # residual, partition-tiled
    nc.sync.dma_start(xr, xr_d.ap().rearrange("p (n c) -> p n c", c=C))
    nc.sync.dma_start(ident, id_d.ap())   # host-provided eye(128)

    # preload the exp activation table while DMAs run (first ACTIVATE of a
    # set pays ~2.7us; do it off the critical path)
    zt = smallp.tile([P, 1], FP32, tag="zt")
    nc.vector.memset(zt, 0.0)
    zo = smallp.tile([P, 1], FP32, tag="zo")
    nc.scalar.activation(zo, zt, AF.Exp)

    # HAM warm-up: real K=128 matmuls with NO input dependencies (both
    # operands read uninitialized SBUF; the psum result is never read) so
    # PE is busy from the instant its queue opens and the 1.2->2.4 GHz
    # un-throttle window elapses before the real work.
    def dummies(n):
        for i in range(n):
            dmy = ps_s.tile([P, SB], FP32, tag="pss", name="dummy")
            nc.tensor.matmul(dmy, lhsT=qt[:, 0:P],
                             rhs=qt[:, i * SB:(i + 1) * SB], start=True, stop=True)

    dummies(4)

    # ---- projections (into explicit ring slots) -------------------------
    def kt_round(g, on_act):  # cols [g*512, (g+1)*512)
        ps = ps_s.tile([P, SB], FP32, tag="pss", name="ktps")
        nc.tensor.matmul(ps, lhsT=wk, rhs=xt[:, g * SB:(g + 1) * SB],
                         start=True, stop=True)
        if on_act:
            nc.scalar.copy(kt[:, g * SB:(g + 1) * SB], ps)
        else:
            nc.vector.tensor_copy(kt[:, g * SB:(g + 1) * SB], ps)

    def qt_round(i):  # cols [i*512, (i+1)*512)
        ps = ps_s.tile([P, SB], FP32, tag="pss", name="qtps")
        nc.tensor.matmul(ps, lhsT=wq, rhs=xt[:, i * SB:(i + 1) * SB],
                         start=True, stop=True)
        nc.vector.tensor_copy(qt[:, i * SB:(i + 1) * SB], ps)

    def va_round(g, on_act):  # s-tiles [g*4, (g+1)*4)
        ps = ps_s.tile([P, 4, P], FP32, tag="pss", name="vaps")
        for j in range(4):
            nc.tensor.matmul(ps[:, j, :], lhsT=xt[:, (g * 4 + j) * P:(g * 4 + j + 1) * P],
                             rhs=wv, start=True, stop=True)
        if on_act:
            nc.scalar.copy(va[:, g * 4:(g + 1) * 4, :], ps)
        else:
            nc.vector.tensor_copy(va[:, g * 4:(g + 1) * 4, :], ps)

    # minimal upfront set: enough for (tb=0, st=0..11).  The first three
    # rounds need only xt cols 0:512 (landed early); more warm-up dummies
    # bridge the wait for cols 512:1024 so the HAM busy-window never breaks.
    kt_round(0, True)
    qt_round(0)
    va_round(0, True)
    dummies(3)
    kt_round(1, True)
    qt_round(1)
    va_round(1, False)

    # remaining setup rounds, interleaved into the tb=0 main loop
    setup_sched = {
        2: [lambda: qt_round(2)],
        4: [lambda: va_round(2, True)],
        6: [lambda: qt_round(3)],
        8: [lambda: va_round(3, False)],
        10: [lambda: qt_round(4)],
        12: [lambda: va_round(4, True)],
        14: [lambda: qt_round(5)],
        16: [lambda: va_round(5, False)],
        18: [lambda: qt_round(6)],
        20: [lambda: va_round(6, True)],
        22: [lambda: qt_round(7)],
        24: [lambda: va_round(7, False)],
        26: [lambda: kt_round(2, True)],
        28: [lambda: kt_round(3, False)],
    }

    # ---- flash attention main loop --------------------------------------
    out_v = out_d.ap().rearrange("p (n c) -> p n c", c=C)  # [128, 16, 64]
    ex = [None] * NT
    po = [None] * N_TB

    def scores(tb, st):
        # matmuls into the two single-bank slots of an aligned pair;
        # one full-width exp over the contiguous [128,1024] pair view
        e = expp.tile([P, TB], BF16, tag="ex", name="ex")
        for h in range(2):
            pss = ps_s.tile([P, SB], FP32, tag="pss", name="pss")
            nc.tensor.matmul(pss,
                             lhsT=qt[:, st * P:(st + 1) * P],
                             rhs=kt[:, tb * TB + h * SB:tb * TB + (h + 1) * SB],
                             start=True, stop=True)
            eh = e[:, h * SB:(h + 1) * SB]
            if st == NT - 1:
                on_dve = h == 1   # endgame: one half on each engine
            else:
                on_dve = st in DVE_EXP
            if on_dve:
                nc.vector.tensor_scalar(eh.bitcast(I16), pss, SCHRAUD_A,
                                        SCHRAUD_B, ALU.mult, ALU.add)
            else:
                nc.scalar.activation(eh, pss, AF.Exp)
        ex[st] = e

    def attn(tb, st):
        for h in range(2):
            nc.tensor.matmul(po[tb][:, h * SB:(h + 1) * SB],
                             lhsT=va[:, st, :],
                             rhs=ex[st][:, h * SB:(h + 1) * SB],
                             start=(st == 0), stop=(st == NT - 1))

    def fin_start(tb):
        # free the shared po region ASAP (tb=1's first attn waits on this)
        osb = osbp.tile([P, TB], BF16, tag="osb")
        nc.scalar.copy(osb[:, 0:SB], po[tb][:, 0:SB])
        nc.vector.tensor_copy(osb[:, SB:TB], po[tb][:, SB:TB])
        return osb

    def fin_chunk(tb, osb, ot, half):
        # transpose 4 chunks via identity matmuls, normalize, add residual;
        # results land in slices of one [P, 8, C] tile so the whole t-block
        # ships as a single contiguous DMA (issue cost is per-DMA, ~700ns
        # serialized on the Sync queue).
        pt = ps_s.tile([P, 4, P], FP32, tag="pss", name="pt")
        for jj in range(4):
            j = half * 4 + jj
            nc.tensor.matmul(pt[:, jj, :], lhsT=osb[:, j * P:(j + 1) * P],
                             rhs=ident, start=True, stop=True)
        recs = []
        for jj in range(4):
            rec = smallp.tile([P, 1], FP32, tag="rec")
            nc.vector.reciprocal(rec, pt[:, jj, C:C + 1])
            recs.append(rec)
        for jj in range(4):
            j = half * 4 + jj
            if jj % 2 == 0:
                nc.scalar.activation(ot[:, j, :], pt[:, jj, 0:C], AF.Copy,
                                     scale=recs[jj])
            else:
                nc.vector.tensor_scalar(ot[:, j, :], pt[:, jj, 0:C], recs[jj],
                                        None, ALU.mult)
        for jj in range(4):
            j = half * 4 + jj
            nc.vector.tensor_add(ot[:, j, :], ot[:, j, :], xr[:, tb * 8 + j, :])
        if half == 1:
            nc.sync.dma_start(out_v[:, tb * 8:(tb + 1) * 8, :], ot)

    fin_sched = {}
    for tb in range(N_TB):
        po[tb] = ps_o.tile([P, TB], FP32, tag="po", name=f"po{tb}")
        for st in range(NT):
            for f in setup_sched.get(st, []) if tb == 0 else fin_sched.get(st, []):
                f()
            scores(tb, st)
            if st >= 2:
                attn(tb, st - 2)
        attn(tb, NT - 2)
        attn(tb, NT - 1)
        osb = fin_start(tb)
        ot = outp.tile([P, 8, C], FP32, tag="ot")
        if tb == 0:
            fin_sched = {3: [lambda: fin_chunk(0, osb, ot, 0)],
                         7: [lambda: fin_chunk(0, osb, ot, 1)]}
        else:
            fin_chunk(tb, osb, ot, 0)
            fin_chunk(tb, osb, ot, 1)


def build():
    nc = bacc.Bacc("TRN2", target_bir_lowering=False, debug=False,
                   num_devices=N_CORES)
    xt_d = nc.dram_tensor("xt", [P, T], BF16, kind="ExternalInput")
    xr_d = nc.dram_tensor("xr", [P, TL // P * C], FP32, kind="ExternalInput")
    wq_d = nc.dram_tensor("wq", [P, P], BF16, kind="ExternalInput")
    wk_d = nc.dram_tensor("wk", [P, P], BF16, kind="ExternalInput")
    wv_d = nc.dram_tensor("wv", [P, P], BF16, kind="ExternalInput")
    id_d = nc.dram_tensor("ident", [P, P], BF16, kind="ExternalInput")
    out_d = nc.dram_tensor("out", [P, TL // P * C], FP32, kind="ExternalOutput")

    with tile.TileContext(nc) as tc, ExitStack() as ctx:
        _emit(tc, ctx, xt_d, xr_d, wq_d, wk_d, wv_d, id_d, out_d)
    nc.compile()
    return nc


def make_in_maps(inputs, Wq, bq, Wk, bk, Wv, bv, gamma):
    """Host-side layout prep + sharding into per-core input maps."""
    bf16 = ml_dtypes.bfloat16
    x = np.asarray(inputs, dtype=np.float32).reshape(B, T, C)
    g = float(np.asarray(gamma, np.float32).reshape(-1)[0])

    def w_aug(W, b, scale=1.0):
        w = np.zeros((P, P), dtype=np.float32)
        w[0:C, 0:C] = np.asarray(W, np.float32) * scale
        w[C, 0:C] = np.asarray(b, np.float32) * scale
        return w.astype(bf16)

    wq_np = w_aug(Wq, bq)
    wk_np = w_aug(Wk, bk)
    wv_np = w_aug(Wv, bv, scale=g)      # gamma folded into V
    wv_np[C, C] = bf16(1.0)             # ones column -> softmax denominator

    in_maps = []
    for core in range(N_CORES):
        b_i, h = divmod(core, HALVES)
        xb = x[b_i]
        if h:
            xb = np.concatenate([xb[h * TL:], xb[:h * TL]], axis=0)
        xt_np = np.zeros((P, T), dtype=bf16)
        xt_np[0:C] = xb.T.astype(bf16)
        xt_np[C] = bf16(1.0)
        xr_np = np.ascontiguousarray(
            xb[0:TL].reshape(TL // P, P, C).transpose(1, 0, 2).reshape(P, -1))
        in_maps.append({
            "xt": xt_np, "xr": xr_np,
            "wq": wq_np, "wk": wk_np, "wv": wv_np,
            "ident": np.eye(P, dtype=bf16),
        })
    return in_maps


def assemble(results):
    """Gather per-core partition-major outputs into the full [B, 1, T, C]."""
    out = np.empty((B, 1, T, C), dtype=np.float32)
    for core in range(N_CORES):
        b_i, h = divmod(core, HALVES)
        dev = np.asarray(results[core]["out"], dtype=np.float32)
        rows = dev.reshape(P, TL // P, C).transpose(1, 0, 2).reshape(TL, C)
        out[b_i, 0, h * TL:(h + 1) * TL, :] = rows
    return out


_NC_CACHE = []


def kernel(inputs, Wq, bq, Wk, bk, Wv, bv, gamma):
    if not _NC_CACHE:
        _NC_CACHE.append(build())
    nc = _NC_CACHE[0]
    in_maps = make_in_maps(inputs, Wq, bq, Wk, bk, Wv, bv, gamma)
    res = run_bass_kernel_spmd(nc, in_maps, list(range(N_CORES)))
    return assemble(res.results)


# revision 29
# speedup vs baseline: 1.0101x; 1.0101x over previous
"""Fused self-attention kernel for Trainium2 (Bass/Tile), SPMD over 8 cores.

Math (per batch b):
    q = x @ Wq + bq ; k = x @ Wk + bk ; v = x @ Wv + bv          [T, C]
    scores[t, s] = k[t] . q[s]      (non-causal, unscaled)
    beta = softmax(scores, axis=s)
    attn[t] = sum_s beta[t, s] * v[s]
    out = gamma * attn + x

Sharding: 8 cores = 4 batches x 2 halves of the output rows t. Each core's
x is rotated so its local 2048 output rows come first (softmax over s is
permutation invariant, so rotating s is safe).

Host-side layout prep (inside kernel(), pure numpy): x is transposed,
ones-augmented and cast to bf16 (xt), the residual slice is pre-tiled
to the SBUF partition layout (xr), and the weights are padded to
[128,128] bf16 with the bias folded in as row 64 (the ones row of xt
multiplies it back in).  gamma is folded into Wv/bv (the ones column of
v_aug that produces the softmax denominator is NOT scaled), so the
device kernel never sees gamma and out = num/denom + x directly.

On-chip: scoresT[s, t] with s on partitions; denominator via the ones
column of v_aug.  No max-subtraction (|scores| < ~60 for normalized
inputs; exp in fp32, overflow at 88).  exp is split between the Scalar
engine (table exp) and the Vector engine (Schraudolph-style bf16 bit
trick: bitcast(round(x*184.665 + 16248.7)) ~= exp(x) to ~3%), since ACT
alone (1 elem/cycle/lane) would be the bottleneck.  The TxT score
matrix never touches HBM.
"""

import numpy as np
import ml_dtypes
from contextlib import ExitStack

import concourse.bass as bass
import concourse.tile as tile
from concourse import bacc, mybir
from concourse.bass_utils import run_bass_kernel_spmd

FP32 = mybir.dt.float32
BF16 = mybir.dt.bfloat16
I16 = mybir.dt.int16
AF = mybir.ActivationFunctionType
ALU = mybir.AluOpType

B, T, C = 4, 4096, 64
P = 128
HALVES = 2
N_CORES = B * HALVES
TL = T // HALVES      # local output rows per core (2048)
TB = 1024             # t-block width (one PSUM accumulator pair)
N_TB = TL // TB       # 2
SB = 512              # psum-bank-sized matmul free dim
NT = T // P           # 32 s-tiles

# Schraudolph exp in bf16 bits: exp(x) ~= bitcast_bf16(round(A*x + BB))
SCHRAUD_A = 128.0 / np.log(2.0)          # 184.6650
SCHRAUD_B = 16256.0 - 0.0573 * 128.0     # 16248.67 (balanced max rel err ~3%)

# which s-tiles' exp goes to the Vector engine (rest on Scalar engine);
# tuned so ACT (997ns/tile + finalize scales) and DVE (1224ns/tile +
# projection copies + finalize) land at similar totals, both under PE
DVE_EXP = set(range(1, NT, 3)) | {2, 14, 26, 30}  # 14 of 32 per t-block


def _emit(tc, ctx, xt_d, xr_d, wq_d, wk_d, wv_d, id_d, out_d):
    nc = tc.nc

    const = ctx.enter_context(tc.tile_pool(name="const", bufs=1))
    expp = ctx.enter_context(tc.tile_pool(name="expp", bufs=6))
    osbp = ctx.enter_context(tc.tile_pool(name="osbp", bufs=2))
    outp = ctx.enter_context(tc.tile_pool(name="outp", bufs=2))
    smallp = ctx.enter_context(tc.tile_pool(name="smallp", bufs=8))
    # PSUM (8 banks): a manually-rotated ring of 6 one-bank slots inside a
    # single [128, 6, 512] tile (scores matmuls target single-bank slots;
    # exp reads a full-width [128,1024] view over an ALIGNED slot pair, so
    # the per-instruction overhead of ACT/DVE is paid once per 1024, not
    # per 512).  Slots advance in pairs; Tile's range-based dependency
    # tracking gives the WAR serialization for slot reuse.  Scores run 2-3
    # s-tiles ahead of attn so exp latency never stalls the PE FIFO.  One
    # shared po accumulator region = 2 banks (tb=0/tb=1 are sequential).
    ps_s = ctx.enter_context(tc.tile_pool(name="ps_s", bufs=6, space="PSUM"))
    ps_o = ctx.enter_context(tc.tile_pool(name="ps_o", bufs=1, space="PSUM"))

    # ---- constants & DMAs ----------------------------------------------
    ident = const.tile([P, P], BF16, tag="ident")
    wq = const.tile([P, P], BF16, tag="wq")
    wk = const.tile([P, P], BF16, tag="wk")
    wv = const.tile([P, P], BF16, tag="wv")
    xt = const.tile([P, T], BF16, tag="xt")   # rows 0:64 x.T, 64 ones, 65: zeros
    qt = const.tile([P, T], BF16, tag="qt")       # q.T, all s
    kt = const.tile([P, TL], BF16, tag="kt")      # k.T, local t
    va = const.tile([P, NT, P], BF16, tag="va")   # v_aug per s-tile [s,c]

    # DMA order = first-use order: xt chunk 0 + wk feed kt_round(0), then
    # wq/wv for the other projections; the identity (finalize-only) and the
    # residual (finalize-only) go last.
    nc.sync.dma_start(xt[:, 0:SB], xt_d.ap()[:, 0:SB])
    nc.sync.dma_start(wk, wk_d.ap())
    nc.sync.dma_start(wq, wq_d.ap())
    nc.sync.dma_start(wv, wv_d.ap())
    nc.sync.dma_start(xt[:, SB:TB], xt_d.ap()[:, SB:TB])
    for i in range(1, 4):
        nc.sync.dma_start(xt[:, i * TB:(i + 1) * TB],
                          xt_d.ap()[:, i * TB:(i + 1) * TB])
    xr = const.tile([P, TL // P, C], FP32, tag="xr")  # residual, partition-tiled
    nc.sync.dma_start(xr, xr_d.ap().rearrange("p (n c) -> p n c", c=C))
    nc.sync.dma_start(ident, id_d.ap())   # host-provided eye(128)

    # preload the exp activation table while DMAs run (first ACTIVATE of a
    # set pays ~2.7us; do it off the critical path)
    zt = smallp.tile([P, 1], FP32, tag="zt")
    nc.vector.memset(zt, 0.0)
    zo = smallp.tile([P, 1], FP32, tag="zo")
    nc.scalar.activation(zo, zt, AF.Exp)

    # HAM warm-up: real K=128 matmuls with NO input dependencies (both
    # operands read uninitialized SBUF; the psum result is never read) so
    # PE is busy from the instant its queue opens and the 1.2->2.4 GHz
    # un-throttle window elapses before the real work.
    def dummies(n):
        for i in range(n):
            dmy = ps_s.tile([P, SB], FP32, tag="pss", name="dummy")
            nc.tensor.matmul(dmy, lhsT=qt[:, 0:P],
                             rhs=qt[:, i * SB:(i + 1) * SB], start=True, stop=True)

    dummies(6)

    # ---- projections (into explicit ring slots) -------------------------
    def kt_round(g, on_act):  # cols [g*512, (g+1)*512)
        ps = ps_s.tile([P, SB], FP32, tag="pss", name="ktps")
        nc.tensor.matmul(ps, lhsT=wk, rhs=xt[:, g * SB:(g + 1) * SB],
                         start=True, stop=True)
        if on_act:
            nc.scalar.copy(kt[:, g * SB:(g + 1) * SB], ps)
        else:
            nc.vector.tensor_copy(kt[:, g * SB:(g + 1) * SB], ps)

    def qt_round(i):  # cols [i*512, (i+1)*512)
        ps = ps_s.tile([P, SB], FP32, tag="pss", name="qtps")
        nc.tensor.matmul(ps, lhsT=wq, rhs=xt[:, i * SB:(i + 1) * SB],
                         start=True, stop=True)
        nc.vector.tensor_copy(qt[:, i * SB:(i + 1) * SB], ps)

    def va_round(g, on_act):  # s-tiles [g*4, (g+1)*4)
        ps = ps_s.tile([P, 4, P], FP32, tag="pss", name="vaps")
        for j in range(4):
            nc.tensor.matmul(ps[:, j, :], lhsT=xt[:, (g * 4 + j) * P:(g * 4 + j + 1) * P],
                             rhs=wv, start=True, stop=True)
        if on_act:
            nc.scalar.copy(va[:, g * 4:(g + 1) * 4, :], ps)
        else:
            nc.vector.tensor_copy(va[:, g * 4:(g + 1) * 4, :], ps)

    # minimal upfront set: enough for (tb=0, st=0..11).  The first three
    # rounds need only xt cols 0:512 (landed early); more warm-up dummies
    # bridge the wait for cols 512:1024 so the HAM busy-window never breaks.
    kt_round(0, True)
    qt_round(0)
    va_round(0, True)
    dummies(4)
    kt_round(1, True)
    qt_round(1)
    va_round(1, False)

    # remaining setup rounds, interleaved into the tb=0 main loop
    setup_sched = {
        2: [lambda: qt_round(2)],
        4: [lambda: va_round(2, True)],
        6: [lambda: qt_round(3)],
        8: [lambda: va_round(3, False)],
        10: [lambda: qt_round(4)],
        12: [lambda: va_round(4, True)],
        14: [lambda: qt_round(5)],
        16: [lambda: va_round(5, False)],
        18: [lambda: qt_round(6)],
        20: [lambda: va_round(6, True)],
        22: [lambda: qt_round(7)],
        24: [lambda: va_round(7, False)],
        26: [lambda: kt_round(2, True)],
        28: [lambda: kt_round(3, False)],
    }

    # ---- flash attention main loop --------------------------------------
    out_v = out_d.ap().rearrange("p (n c) -> p n c", c=C)  # [128, 16, 64]
    ex = [None] * NT
    po = [None] * N_TB

    def scores(tb, st):
        # matmuls into the two single-bank slots of an aligned pair;
        # one full-width exp over the contiguous [128,1024] pair view
        e = expp.tile([P, TB], BF16, tag="ex", name="ex")
        for h in range(2):
            pss = ps_s.tile([P, SB], FP32, tag="pss", name="pss")
            nc.tensor.matmul(pss,
                             lhsT=qt[:, st * P:(st + 1) * P],
                             rhs=kt[:, tb * TB + h * SB:tb * TB + (h + 1) * SB],
                             start=True, stop=True)
            eh = e[:, h * SB:(h + 1) * SB]
            if st == NT - 1:
                on_dve = h == 1   # endgame: one half on each engine
            else:
                on_dve = st in DVE_EXP
            if on_dve:
                nc.vector.tensor_scalar(eh.bitcast(I16), pss, SCHRAUD_A,
                                        SCHRAUD_B, ALU.mult, ALU.add)
            else:
                nc.scalar.activation(eh, pss, AF.Exp)
        ex[st] = e

    def attn(tb, st):
        for h in range(2):
            nc.tensor.matmul(po[tb][:, h * SB:(h + 1) * SB],
                             lhsT=va[:, st, :],
                             rhs=ex[st][:, h * SB:(h + 1) * SB],
                             start=(st == 0), stop=(st == NT - 1))

    def fin_start(tb):
        # free the shared po region ASAP (tb=1's first attn waits on this)
        osb = osbp.tile([P, TB], BF16, tag="osb")
        nc.scalar.copy(osb[:, 0:SB], po[tb][:, 0:SB])
        nc.vector.tensor_copy(osb[:, SB:TB], po[tb][:, SB:TB])
        return osb

    def fin_chunk(tb, osb, ot, half):
        # transpose 4 chunks via identity matmuls, normalize, add residual;
        # results land in slices of one [P, 8, C] tile so the whole t-block
        # ships as a single contiguous DMA (issue cost is per-DMA, ~700ns
        # serialized on the Sync queue).
        pt = ps_s.tile([P, 4, P], FP32, tag="pss", name="pt")
        for jj in range(4):
            j = half * 4 + jj
            nc.tensor.matmul(pt[:, jj, :], lhsT=osb[:, j * P:(j + 1) * P],
                             rhs=ident, start=True, stop=True)
        recs = []
        for jj in range(4):
            rec = smallp.tile([P, 1], FP32, tag="rec")
            nc.vector.reciprocal(rec, pt[:, jj, C:C + 1])
            recs.append(rec)
        for jj in range(4):
            j = half * 4 + jj
            if jj % 2 == 0:
                nc.scalar.activation(ot[:, j, :], pt[:, jj, 0:C], AF.Copy,
                                     scale=recs[jj])
            else:
                nc.vector.tensor_scalar(ot[:, j, :], pt[:, jj, 0:C], recs[jj],
                                        None, ALU.mult)
        for jj in range(4):
            j = half * 4 + jj
            nc.vector.tensor_add(ot[:, j, :], ot[:, j, :], xr[:, tb * 8 + j, :])
        if half == 1:
            nc.sync.dma_start(out_v[:, tb * 8:(tb + 1) * 8, :], ot)

    fin_sched = {}
    for tb in range(N_TB):
        po[tb] = ps_o.tile([P, TB], FP32, tag="po", name=f"po{tb}")
        for st in range(NT):
            for f in setup_sched.get(st, []) if tb == 0 else fin_sched.get(st, []):
                f()
            scores(tb, st)
            if st >= 2:
                attn(tb, st - 2)
        attn(tb, NT - 2)
        attn(tb, NT - 1)
        osb = fin_start(tb)
        ot = outp.tile([P, 8, C], FP32, tag="ot")
        if tb == 0:
            fin_sched = {3: [lambda: fin_chunk(0, osb, ot, 0)],
                         7: [lambda: fin_chunk(0, osb, ot, 1)]}
        else:
            fin_chunk(tb, osb, ot, 0)
            fin_chunk(tb, osb, ot, 1)


def build():
    nc = bacc.Bacc("TRN2", target_bir_lowering=False, debug=False,
                   num_devices=N_CORES)
    xt_d = nc.dram_tensor("xt", [P, T], BF16, kind="ExternalInput")
    xr_d = nc.dram_tensor("xr", [P, TL // P * C], FP32, kind="ExternalInput")
    wq_d = nc.dram_tensor("wq", [P, P], BF16, kind="ExternalInput")
    wk_d = nc.dram_tensor("wk", [P, P], BF16, kind="ExternalInput")
    wv_d = nc.dram_tensor("wv", [P, P], BF16, kind="ExternalInput")
    id_d = nc.dram_tensor("ident", [P, P], BF16, kind="ExternalInput")
    out_d = nc.dram_tensor("out", [P, TL // P * C], FP32, kind="ExternalOutput")

    with tile.TileContext(nc) as tc, ExitStack() as ctx:
        _emit(tc, ctx, xt_d, xr_d, wq_d, wk_d, wv_d, id_d, out_d)
    nc.compile()
    return nc


def make_in_maps(inputs, Wq, bq, Wk, bk, Wv, bv, gamma):
    """Host-side layout prep + sharding into per-core input maps."""
    bf16 = ml_dtypes.bfloat16
    x = np.asarray(inputs, dtype=np.float32).reshape(B, T, C)
    g = float(np.asarray(gamma, np.float32).reshape(-1)[0])

    def w_aug(W, b, scale=1.0):
        w = np.zeros((P, P), dtype=np.float32)
        w[0:C, 0:C] = np.asarray(W, np.float32) * scale
        w[C, 0:C] = np.asarray(b, np.float32) * scale
        return w.astype(bf16)

    wq_np = w_aug(Wq, bq)
    wk_np = w_aug(Wk, bk)
    wv_np = w_aug(Wv, bv, scale=g)      # gamma folded into V
    wv_np[C, C] = bf16(1.0)             # ones column -> softmax denominator

    in_maps = []
    for core in range(N_CORES):
        b_i, h = divmod(core, HALVES)
        xb = x[b_i]
        if h:
            xb = np.concatenate([xb[h * TL:], xb[:h * TL]], axis=0)
        xt_np = np.zeros((P, T), dtype=bf16)
        xt_np[0:C] = xb.T.astype(bf16)
        xt_np[C] = bf16(1.0)
        xr_np = np.ascontiguousarray(
            xb[0:TL].reshape(TL // P, P, C).transpose(1, 0, 2).reshape(P, -1))
        in_maps.append({
            "xt": xt_np, "xr": xr_np,
            "wq": wq_np, "wk": wk_np, "wv": wv_np,
            "ident": np.eye(P, dtype=bf16),
        })
    return in_maps


def assemble(results):
    """Gather per-core partition-major outputs into the full [B, 1, T, C]."""
    out = np.empty((B, 1, T, C), dtype=np.float32)
    for core in range(N_CORES):
        b_i, h = divmod(core, HALVES)
        dev = np.asarray(results[core]["out"], dtype=np.float32)
        rows = dev.reshape(P, TL // P, C).transpose(1, 0, 2).reshape(TL, C)
        out[b_i, 0, h * TL:(h + 1) * TL, :] = rows
    return out


_NC_CACHE = []


def kernel(inputs, Wq, bq, Wk, bk, Wv, bv, gamma):
    if not _NC_CACHE:
        _NC_CACHE.append(build())
    nc = _NC_CACHE[0]
    in_maps = make_in_maps(inputs, Wq, bq, Wk, bk, Wv, bv, gamma)
    res = run_bass_kernel_spmd(nc, in_maps, list(range(N_CORES)))
    return assemble(res.results)
